# revision 74
# baseline (speedup 1.0000x reference)
"""Trainium2 Bass kernel for nn_EncoderBlock (dual self-attention + BN + FFN + BN).

Sharding: data-parallel over batch (16 batches -> 2 per core on 8 cores).
Device layout: activations transposed (channels E on partitions, tokens on free
dim) so BatchNorm stats are free-dim reductions. Attention computes transposed
scores sT[h] = k_h @ q_h.T so softmax needs no on-device transposes; a ones
column appended to V produces softmax denominators inside the AV matmul; the
per-query reciprocal denominators are broadcast across partitions with a tiny
K=2 matmul.

Precision plan (rel-err budget 2e-2, lands ~1e-2):
- All weight matmuls (QKV/O projections, FFN) run in fp8e4m3 with DoubleRow
  perf mode: two 128-deep contraction tiles per instruction at 0.5 PE
  cycles/row. Weights are pre-quantized on the host in [128, 2, E] k-tile-pair
  layout; activations are quantized on the fly during PSUM evacuation.
- Scores stay f32r (contraction is dk=64: DoubleRow's k-tile pairing cannot
  apply, and fp8 without DoubleRow has no rate advantage).
- exp(scores) is written directly as fp8 and consumed by a DoubleRow AV
  matmul against fp8 V (ones column included for denominators).
- The residual x is added inside the output-projection PSUM group via a
  bf16 identity matmul (x is loaded bf16 only; f32 x is never needed).
- Output is written bf16 and upcast on the host.

Algebraic eliminations (exact): K-projection bias is softmax-invariant
(q+bq)@(k+bk) == (q+bq)@k + per-query const; V/O/FFN2 biases and the BN1
shift in the FFN2 residual are per-channel constants which the following
BatchNorm subtracts out exactly. Only the Q bias, FFN1 bias (inside relu),
and the BN affine outputs are applied.

BatchNorm batch stats use a 4KB AllReduce across the 8 cores (twice); in
for_timing builds the collective is replaced by a same-shape DRAM copy
(TimelineSim cannot model collectives; the real kernel runs the AllReduce).
"""

import numpy as np
import concourse.bass as bass
import concourse.bacc as bacc
import concourse.tile as tile
from concourse import mybir
from concourse.bass_utils import run_bass_kernel_spmd

dt = mybir.dt
F32 = dt.float32
F32R = dt.float32r
BF16 = dt.bfloat16
F8 = dt.float8e4
AF = mybir.ActivationFunctionType
OP = mybir.AluOpType
DR = mybir.MatmulPerfMode.DoubleRow

N_CORES = 8
B, N, E, H, DK = 16, 1024, 512, 8, 64
NR, NT = 256, 768          # robot / task sequence lengths
BL = B // N_CORES          # local batches per core
TOK = BL * N               # local tokens per core
EC = E // 128              # channel chunks of 128
N_GLOBAL = B * N           # BN stat count
EPS = 1e-5

W_NAMES = ["rq", "rk", "rv", "ro", "tq", "tk", "tv", "to", "f1", "f2"]
B_NAMES = ["rq", "tq", "f1", "bn1_g", "bn1_b", "bn2_g", "bn2_b"]


def _bank_slices(base, length):
    """Split [base, base+length) into pieces (<=512) that never cross a
    512-col PSUM bank boundary. base/length multiples of 256."""
    out = []
    cur = base
    end = base + length
    while cur < end:
        nb = (cur // 512 + 1) * 512
        fl = min(end, nb) - cur
        out.append((cur - base, fl))
        cur += fl
    return out


def build(for_timing=False):
    nc = bacc.Bacc("TRN2", target_bir_lowering=False, debug=False,
                   num_devices=N_CORES)

    x8_d = nc.dram_tensor("x8", [128, 2, 2, TOK], F8, kind="ExternalInput")
    xb_d = nc.dram_tensor("xb", [E, TOK], BF16, kind="ExternalInput")
    w_d = {n: nc.dram_tensor(f"{n}_w8", [128, 2, 2, E], F8,
                             kind="ExternalInput")
           for n in W_NAMES}
    bpk_d = nc.dram_tensor("bpk", [128, len(B_NAMES) * EC], F32,
                           kind="ExternalInput")
    sel2_d = nc.dram_tensor("sel2", [98, 128], BF16,
                            kind="ExternalInput")
    id_d = nc.dram_tensor("idb", [128, 128], BF16, kind="ExternalInput")
    ones_d = nc.dram_tensor("ones8", [128, 3 * 2 * H * 2], F8,
                             kind="ExternalInput")
    yT_d = nc.dram_tensor("yT", [E, TOK], BF16, kind="ExternalOutput")

    from contextlib import ExitStack
    with tile.TileContext(nc) as tc, ExitStack() as es:
        const = es.enter_context(tc.tile_pool(name="const", bufs=1))
        wpool = es.enter_context(tc.tile_pool(name="w", bufs=1))
        act = es.enter_context(tc.tile_pool(name="act", bufs=1))
        attn = es.enter_context(tc.tile_pool(name="attn", bufs=1))
        expp = es.enter_context(tc.tile_pool(name="expp", bufs=2))
        small = es.enter_context(tc.tile_pool(name="small", bufs=2))
        dram = es.enter_context(tc.tile_pool(name="dram", bufs=1, space="DRAM"))
        ps_big = es.enter_context(tc.tile_pool(name="ps_big", bufs=2, space="PSUM"))
        ps_av = es.enter_context(tc.tile_pool(name="ps_av", bufs=2, space="PSUM"))
        _body(nc, const, wpool, act, attn, expp, small, dram, ps_big, ps_av,
              x8_d, xb_d, w_d, bpk_d, sel2_d, id_d, ones_d, yT_d, for_timing)
    nc.finalize()
    return nc


def _load_w8(nc, wpool, w_dram, tag, queue=None):
    """fp8 weight matrix as one [128, 2, 2, E] tile (g-major k-tile pairs),
    loaded in a single DMA."""
    t = wpool.tile([128, 2, 2, E], F8, tag=tag, name=tag)
    (queue or nc.sync).dma_start(out=t[:], in_=w_dram.ap())
    return t


class _Ctx:
    pass


def _body(nc, const, wpool, act, attn, expp, small, dram, ps_big, ps_av,
          x8_d, xb_d, w_d, bpk_d, sel2_d, id_d, ones_d, yT_d, for_timing):
    # ---------- constants / inputs resident in SBUF ----------
    # DMA emission order == HWDGE issue order: the first projection needs
    # bpk + wq + x8 robot-b0 columns; everything else trickles in after
    bpk = const.tile([128, len(B_NAMES) * EC], F32, tag="bpk", name="bpk")
    nc.sync.dma_start(out=bpk[:], in_=bpk_d.ap())
    bias = {n: bpk[:, i * EC:(i + 1) * EC] for i, n in enumerate(B_NAMES)}

    # latency-critical robot-phase loads on the SP HWDGE queue (the fused
    # robot part runs first; the task part follows)
    w8 = {"rq": _load_w8(nc, wpool, w_d["rq"], "wrq")}
    x8 = [const.tile([128, 2, TOK], F8, tag=f"x8_{g}", name=f"x8_{g}")
          for g in range(2)]
    for g in range(2):
        nc.sync.dma_start(
            out=x8[g][:].rearrange("p i (b n) -> p i b n", b=2)[:, :, :, :NR],
            in_=x8_d.ap()[:, g, :, :].rearrange(
                "p i (b n) -> p i b n", b=2)[:, :, :, :NR])
    w8["rk"] = _load_w8(nc, wpool, w_d["rk"], "wrk")
    w8["rv"] = _load_w8(nc, wpool, w_d["rv"], "wrv")
    sel2 = const.tile([98, 128], BF16, tag="sel2", name="sel2")
    nc.sync.dma_start(out=sel2[:], in_=sel2_d.ap())
    idb = const.tile([128, 128], BF16, tag="idb", name="idb")
    nc.sync.dma_start(out=idb[:], in_=id_d.ap())
    w8["ro"] = _load_w8(nc, wpool, w_d["ro"], "wro")
    xb = [const.tile([128, TOK], BF16, tag=f"xb{m}", name=f"xb{m}")
          for m in range(EC)]
    for m in range(EC):
        nc.sync.dma_start(
            out=xb[m][:].rearrange("p (b n) -> p b n", b=2)[:, :, :NR],
            in_=xb_d.ap()[m * 128:(m + 1) * 128, :].rearrange(
                "p (b n) -> p b n", b=2)[:, :, :NR])

    # persistent fp8 V tiles, one per batch parity; the ones column
    # (softmax denominator trick) is initialized once, first in the SWDGE
    # queue so robot AV is never blocked on it
    v_all = [attn.tile([128, 3, 2, H, DK + 4], F8, tag=f"v_{par}",
                       name=f"v_{par}") for par in range(2)]
    for par in range(2):
        nc.gpsimd.dma_start(
            out=v_all[par][:, :, :, :, DK:DK + 2],
            in_=ones_d.ap().rearrange("p (t i h one) -> p t i h one",
                                      t=3, i=2, one=2))

    # bulk task/FFN loads go through the Pool SWDGE queue so they never sit
    # in front of the latency-critical robot transfers on SP, and so their
    # issue overhead stays off the SP sequencer
    for g in range(2):
        nc.gpsimd.dma_start(
            out=x8[g][:].rearrange("p i (b n) -> p i b n", b=2)[:, :, :, NR:],
            in_=x8_d.ap()[:, g, :, :].rearrange(
                "p i (b n) -> p i b n", b=2)[:, :, :, NR:])
    for nm in ["tq", "tk", "tv", "to"]:
        w8[nm] = _load_w8(nc, wpool, w_d[nm], f"w{nm}", queue=nc.gpsimd)
    for m in range(EC):
        nc.gpsimd.dma_start(
            out=xb[m][:].rearrange("p (b n) -> p b n", b=2)[:, :, NR:],
            in_=xb_d.ap()[m * 128:(m + 1) * 128, :].rearrange(
                "p (b n) -> p b n", b=2)[:, :, NR:])
    for nm in ["f1", "f2"]:
        w8[nm] = _load_w8(nc, wpool, w_d[nm], f"w{nm}", queue=nc.gpsimd)

    # prefetch the exp ACT table set while input DMAs are in flight
    warm = const.tile([1, 1], F32, tag="warm", name="warm")
    nc.vector.memset(warm[:], 0.0)
    nc.scalar.activation(out=warm[:], in_=warm[:], func=AF.Exp, scale=1.0)

    # h-tilde (pre-BN1 attention output) accumulated across parts/batches
    ht = [act.tile([128, TOK], F32, tag=f"ht{k}", name=f"ht{k}")
          for k in range(EC)]

    # ---------- attention (emission software-pipelined) ----------
    # The task part (Np=768) runs per batch; the robot part (Np=256) fuses
    # both batches into one block-diagonal 512-column stream (b*256+t) so
    # its latency chains are half as many and twice as wide.
    def _bview(t):
        return t.rearrange("p (b n) -> p b n", b=2)

    def proj_steps(P, b):
        """Allocate per-batch projection state and return (st, steps):
        emission thunks, one per channel chunk, each emitting the q/k
        projections for chunk m plus the scheduled V chunks."""
        st = _Ctx()
        st.tok0 = b * N + NR if not P.fused else None
        st.b = b
        st.qT = [attn.tile([128, NT], BF16, tag=f"qT{m}_{b}", name=f"qT{m}_{b}")
                 for m in range(EC)]
        st.kT = [attn.tile([128, NT], BF16, tag=f"kT{m}_{b}", name=f"kT{m}_{b}")
                 for m in range(EC)]
        st.ex = [None] * H
        if P.fused:
            st.zT = [attn.tile([128, 2 * NR], BF16, tag=f"zTr{p}",
                               name=f"zTr{p}") for p in range(4)]
            st.z8 = [attn.tile([128, 2, 2 * NR], F8, tag=f"z8r_{g}",
                               name=f"z8r_{g}") for g in range(2)]
            st.rows = small.tile([98, 2 * NR], BF16, tag="rowsr",
                                 name="rowsr", bufs=1)
            st.rinv = small.tile([98, 2 * NR], BF16, tag="rinvr",
                                 name="rinvr", bufs=1)
        else:
            st.zT = [attn.tile([128, NT], BF16, tag=f"zT{p}", name=f"zT{p}")
                     for p in range(4)]
            st.z8 = [attn.tile([128, 2, NT], F8, tag=f"z8_{g}_{b}",
                               name=f"z8_{g}_{b}") for g in range(2)]
            st.rows = small.tile([98, NT], BF16, tag="rows", name="rows",
                                 bufs=1)
            st.rinv = small.tile([98, NT], BF16, tag="rinv", name="rinv",
                                 bufs=1)

        def emit_qk(w_t, o_t, m, with_bias):
            # projection psums live in the av-tag slot: their DVE-paced
            # evacuations must never gate the score-psum rotation
            ps = ps_av.tile([128, NT], F32, tag="av", name="psq", bufs=1)
            wsl = w_t[:, :, :, m * 128:(m + 1) * 128]
            if P.fused:
                nc.tensor.matmul(
                    _bview(ps[:, 0:P.np]),
                    wsl[:, 0, :, :],
                    x8[0][:].rearrange("p i (b n) -> p i b n",
                                       b=2)[:, :, :, 0:NR],
                    start=True, stop=False, perf_mode=DR)
                nc.tensor.matmul(
                    _bview(ps[:, 0:P.np]),
                    wsl[:, 1, :, :],
                    x8[1][:].rearrange("p i (b n) -> p i b n",
                                       b=2)[:, :, :, 0:NR],
                    start=False, stop=True, perf_mode=DR)
            else:
                for off, fl in _bank_slices(0, P.np):
                    for g in range(2):
                        nc.tensor.matmul(
                            ps[:, off:off + fl], wsl[:, g, :, :],
                            x8[g][:, :, st.tok0 + off:st.tok0 + off + fl],
                            start=(g == 0), stop=(g == 1), perf_mode=DR)
            with nc.allow_low_precision(reason="bf16 attn operands"):
                if with_bias and P.fused:
                    # robot-phase Q evacs ride the otherwise-idle Act engine
                    nc.scalar.activation(
                        out=o_t[m][:, 0:P.np], in_=ps[:, 0:P.np],
                        func=AF.Identity, bias=bias[P.wn[0]][:, m:m + 1],
                        scale=1.0)
                elif with_bias:
                    nc.vector.tensor_scalar(
                        out=o_t[m][:, 0:P.np], in0=ps[:, 0:P.np],
                        scalar1=bias[P.wn[0]][:, m:m + 1], scalar2=None,
                        op0=OP.add)
                else:
                    # K bias is softmax-invariant: plain evacuation
                    nc.vector.tensor_copy(out=o_t[m][:, 0:P.np],
                                          in_=ps[:, 0:P.np])

        def emit_v(t):
            # V bias is erased by BN1: plain quantizing evacuation
            # (GPSIMD cannot read PSUM, so this stays on DVE)
            if P.fused:
                vb, tt = t // 2, t % 2
                tok = vb * N + tt * 128
            else:
                vb, tt = b, None
                tok = st.tok0 + t * 128
            # robot V psums ride the sc rotation: their evacuations feed the
            # deferred AZ units, not the latency-critical q/k->score chain
            if P.fused:
                ps = ps_big.tile([128, E], F32, tag="sc", name="psv")
            else:
                ps = ps_av.tile([128, E], F32, tag="av", name="psv", bufs=1)
            for g in range(2):
                nc.tensor.matmul(
                    ps[:], x8[g][:, :, tok:tok + 128],
                    P.wv[:, g, :, :], start=(g == 0), stop=(g == 1),
                    perf_mode=DR)
            slot = (0, tt) if P.fused else (t // 2, t % 2)
            with nc.allow_low_precision(reason="fp8 AV operands"):
                nc.vector.tensor_copy(
                    out=v_all[vb][:, slot[0], slot[1], :, 0:DK],
                    in_=ps[:].rearrange("p (h d) -> p h d", h=H))

        v_sched = ([[0, 1], [2, 3], [4], [5]] if not P.fused
                   else [[0, 1], [2, 3], [], []])

        def step(m):
            emit_qk(P.wq, st.qT, m, True)
            emit_qk(P.wk, st.kT, m, False)
            for t in v_sched[m]:
                emit_v(t)

        return st, [lambda m=m: step(m) for m in range(EC)]

    def heads_se(P, st, h):
        """Scores + fp8 exp for one head."""
        Np = P.np
        qh = st.qT[h // 2][(h % 2) * 64:(h % 2) * 64 + 64, 0:Np]
        kh = st.kT[h // 2][(h % 2) * 64:(h % 2) * 64 + 64, 0:Np]
        st.ex[h] = []
        if P.fused:
            sc = ps_big.tile([128, 2, NT], F32, tag="sc", name="sc")
            for vb in range(2):
                for i in range(2):
                    nc.tensor.matmul(
                        sc[:, i, vb * 256:(vb + 1) * 256],
                        kh[:, vb * 256 + i * 128:vb * 256 + (i + 1) * 128],
                        qh[:, vb * 256:(vb + 1) * 256],
                        start=True, stop=True)
            ex = expp.tile([128, 2, 2 * NR], F8, tag="exr", name="exr",
                           bufs=8)
            with nc.allow_low_precision(reason="fp8 AV operands"):
                nc.scalar.activation(out=ex[:, :, 0:Np], in_=sc[:, :, 0:Np],
                                     func=AF.Exp, scale=0.125)
            st.ex[h].append(ex)
            return
        for grp in range(P.nk // 2):
            sc = ps_big.tile([128, 2, NT], F32, tag="sc", name="sc")
            for j2 in range(2):
                kc = 2 * grp + j2
                for off, fl in _bank_slices(j2 * NT, Np):
                    nc.tensor.matmul(sc[:, j2, off:off + fl],
                                     kh[:, kc * 128:(kc + 1) * 128],
                                     qh[:, off:off + fl],
                                     start=True, stop=True)
            ex = expp.tile([128, 2, NT], F8, tag="exp", name="exp", bufs=10)
            with nc.allow_low_precision(reason="fp8 AV operands"):
                nc.scalar.activation(out=ex[:, :, 0:Np], in_=sc[:, :, 0:Np],
                                     func=AF.Exp, scale=0.125)
            st.ex[h].append(ex)

    def heads_az(P, st, h):
        """DoubleRow AV (+denominator row) for one head; psum evacuated
        partition-aligned then remapped into pair tiles via DMA."""
        Np = P.np
        pair, j = h // 2, h % 2
        zu = ps_av.tile([66, NT], F32, tag="av", name="av", bufs=1)
        if P.fused:
            for vb in range(2):
                nc.tensor.matmul(
                    zu[:, vb * 256:(vb + 1) * 256],
                    v_all[vb][:, 0, :, h, 0:DK + 2],
                    st.ex[h][0][:, :, vb * 256:(vb + 1) * 256],
                    start=True, stop=True, perf_mode=DR)
        else:
            for grp in range(P.nk // 2):
                for off, fl in _bank_slices(0, Np):
                    nc.tensor.matmul(
                        zu[:, off:off + fl],
                        v_all[st.b][:, grp, :, h, 0:DK + 2],
                        st.ex[h][grp][:, :, off:off + fl],
                        start=(grp == 0), stop=(grp == P.nk // 2 - 1),
                        perf_mode=DR)
        st.ex[h] = None
        zst = expp.tile([65, NT], BF16, tag="zst", name="zst", bufs=3)
        zu = zu[0:65, :]
        with nc.allow_low_precision(reason="z is fp8-quantized downstream"):
            if P.fused:
                nc.scalar.activation(out=zst[:, 0:Np], in_=zu[:, 0:Np],
                                     func=AF.Identity, scale=1.0)
            else:
                nc.vector.tensor_copy(out=zst[:, 0:Np], in_=zu[:, 0:Np])
        nc.sync.dma_start(out=st.zT[pair][j * 64:(j + 1) * 64, 0:Np],
                          in_=zst[0:64, 0:Np])
        nc.sync.dma_start(
            out=st.rows[32 * pair + j:32 * pair + j + 1, 0:Np],
            in_=zst[64:65, 0:Np])

    def denom_pair(P, st, pair):
        """Reciprocal of the pair's denominators (deferred a step so the DVE
        queue never stalls on the rowsum DMA), broadcast across partitions
        (K=2 matmul) and folded into z while quantizing to fp8."""
        Np = P.np
        with nc.allow_low_precision(reason="z is fp8-quantized downstream"):
            nc.vector.reciprocal(
                out=st.rinv[32 * pair:32 * pair + 2, 0:Np],
                in_=st.rows[32 * pair:32 * pair + 2, 0:Np])
        rinv = st.rinv[32 * pair:32 * pair + 2, 0:Np]
        rep = ps_av.tile([128, NT], F32, tag="av", name="rep", bufs=1)
        tp = (96, 0) if pair == 3 else None
        for off, fl in _bank_slices(0, Np):
            nc.tensor.matmul(rep[:, off:off + fl],
                             sel2[32 * pair:32 * pair + 2, :],
                             rinv[:, off:off + fl], start=True, stop=True,
                             tile_position=tp)
        with nc.allow_low_precision(reason="fp8 outproj operands"):
            nc.vector.tensor_tensor(out=st.z8[pair // 2][:, pair % 2, 0:Np],
                                    in0=st.zT[pair][:, 0:Np],
                                    in1=rep[:, 0:Np], op=OP.mult)

    def outproj_m(P, st, m, tail=False, on_sc=False):
        """One channel chunk of the DoubleRow output projection of the
        normalized fp8 z, with the residual added via a bf16 identity
        matmul, evacuated into ht with BN1 stats for the newly completed
        256-col slices. `tail` (very last batch) runs on the free score
        psum and evacuates on Act so the BN1 chain is as short as
        possible."""
        Np = P.np
        if tail or on_sc:
            ps = ps_big.tile([128, NT], F32, tag="sc", name="pso")
        else:
            ps = ps_av.tile([128, NT], F32, tag="av", name="pso", bufs=1)
        for off, fl in _bank_slices(0, Np):
            for g in range(2):
                nc.tensor.matmul(ps[:, off:off + fl],
                                 P.wo[:, g, :, m * 128:(m + 1) * 128],
                                 st.z8[g][:, :, off:off + fl],
                                 start=(g == 0), stop=False, perf_mode=DR)
            # residual: += I128 @ x (bf16); O bias is erased by BN1
            if P.fused:
                nc.tensor.matmul(_bview(ps[:, 0:Np]), idb[:],
                                 _bview(xb[m][:])[:, :, 0:NR],
                                 start=False, stop=True)
            else:
                nc.tensor.matmul(ps[:, off:off + fl], idb[:],
                                 xb[m][:, st.tok0 + off:st.tok0 + off + fl],
                                 start=False, stop=True)
            if P.fused:
                break
        if P.fused:
            dst = _bview(xb[m][:])[:, :, 0:NR]
            dst = _bview(ht[m][:])[:, :, 0:NR]
            src_ = _bview(ps[:, 0:Np])
        else:
            dst = ht[m][:, st.tok0:st.tok0 + Np]
            src_ = ps[:, 0:Np]
        if tail:
            nc.scalar.activation(out=dst, in_=src_, func=AF.Identity,
                                 scale=1.0)
        else:
            nc.vector.tensor_copy(out=dst, in_=src_)
        # emit BN1 stats for the 256-col slices this part/batch completes,
        # so only the last slice's stats sit on the BN1 critical path
        for c in P.st_slices[st.b]:
            nc.vector.bn_stats(out=st1_tiles[m][:, c, :],
                               in_=ht[m][:, c * 256:(c + 1) * 256])

    from collections import deque
    backlog = deque()

    def pump():
        if backlog:
            backlog.popleft()()

    def attention_batch(P, st, psteps_next, pumps=2, se_next=None):
        """One batch's AZ stream with denominators folded in; each step also
        emits one deferred unit: the next batch's projections first, then
        whatever is in the backlog (the previous batch's output
        projection)."""
        for h in range(H):
            heads_az(P, st, h)
            if h % 2 == 0 and h > 0:
                denom_pair(P, st, h // 2 - 1)
            if h + 3 < H:
                heads_se(P, st, h + 3)
            elif se_next:
                se_next.popleft()()
            if psteps_next:
                psteps_next.popleft()()
            else:
                for _ in range(pumps):
                    pump()
        denom_pair(P, st, 3)

    st1_tiles = _bn_stats_tiles(small, "bn1", 8)
    st2_tiles = _bn_stats_tiles(small, "bn2", 4)
    f18, f28 = w8["f1"], w8["f2"]

    # ----- robot part first: both batches fused block-diagonally; its
    # scores/exps run up front and everything downstream of the exps is
    # deferred into the task stream via the backlog -----
    R = _Ctx()
    R.part, R.fused = 0, True
    R.wn = ["rq", "rk", "rv", "ro"]
    R.np, R.nk = 2 * NR, 4
    R.st_slices = ([0, 4],)
    R.wq, R.wk, R.wv, R.wo = w8["rq"], w8["rk"], w8["rv"], w8["ro"]

    str_, pr = proj_steps(R, 0)
    for m in range(EC):
        pr[m]()
        heads_se(R, str_, m)
    for h in range(EC, H):
        heads_se(R, str_, h)
    rob = [lambda h=h: heads_az(R, str_, h) for h in range(H)]
    rob[2:2] = [lambda: denom_pair(R, str_, 0)]
    rob[5:5] = [lambda: denom_pair(R, str_, 1)]
    rob[8:8] = [lambda: denom_pair(R, str_, 2)]
    rob.append(lambda: denom_pair(R, str_, 3))
    rob += [lambda m=m: outproj_m(R, str_, m, on_sc=True)
            for m in range(EC)]
    backlog.extend(rob)

    # ----- task part: two batches, software-pipelined -----
    P = _Ctx()
    P.part, P.fused = 1, False
    P.wn = ["tq", "tk", "tv", "to"]
    P.np, P.nk = NT, 6
    P.st_slices = ([1, 2, 3], [5, 6, 7])
    P.wq, P.wk, P.wv, P.wo = w8["tq"], w8["tk"], w8["tv"], w8["to"]

    st0, p0 = proj_steps(P, 0)
    st1, p1 = proj_steps(P, 1)
    p0[0]()
    heads_se(P, st0, 0)
    pump()
    p0[1]()
    heads_se(P, st0, 1)
    pump()
    p0[2]()
    heads_se(P, st0, 2)
    pump()
    p0[3]()
    pump()
    attention_batch(P, st0, deque(p1), pumps=1,
                    se_next=deque([lambda h=h: heads_se(P, st1, h)
                                   for h in range(3)]))
    attention_batch(P, st1, None, pumps=1)
    while backlog:
        pump()
    # all exps done: swap the ACT table set to sqrt (the set also holds
    # relu/identity for the FFN and BN2 phases)
    warm2 = const.tile([1, 1], F32, tag="warm", name="warm2")
    nc.vector.memset(warm2[:], 1.0)
    nc.scalar.activation(out=warm2[:], in_=warm2[:], func=AF.Sqrt, scale=1.0)
    # both task batches' output projections as one wide tail: PE back-to-back
    # on the now-free score psum, Act evacuations in parallel with DVE stats
    for m in range(EC):
        outproj_m(P, st0, m, tail=True)
    for m in range(EC):
        outproj_m(P, st1, m, tail=True)

    # ---------- BN1 ----------
    s1, t1 = _bn_params(nc, small, dram, st1_tiles, bias["bn1_g"],
                        bias["bn1_b"], "bn1", for_timing)

    # ---------- FFN (token-slice pipelined; BN1 folded into the fp8
    # quantization of ht and into the FFN2 epilogue) ----------
    ht8 = [act.tile([128, 2, TOK], F8, tag=f"ht8_{g}", name=f"ht8_{g}")
           for g in range(2)]
    h18 = [act.tile([128, 2, TOK], F8, tag=f"h18_{g}", name=f"h18_{g}")
           for g in range(2)]
    ho = [act.tile([128, TOK], F32, tag=f"ho{k}", name=f"ho{k}")
          for k in range(EC)]
    FSL = [(0, 512, [0]), (512, 1024, [1, 2]), (1536, 512, [3])]
    deferred_stats = []
    for si, (o0, ln, grps) in enumerate(FSL):
        sl = slice(o0, o0 + ln)
        # quantize BN1(ht) for this token slice (first slice on the
        # just-idle DVE so FFN1 starts immediately; rest on gpsimd)
        with nc.allow_low_precision(reason="fp8 FFN operands"):
            for m in range(EC):
                eng = nc.vector if si == 0 else nc.gpsimd
                eng.tensor_scalar(
                    out=ht8[m // 2][:, m % 2, sl], in0=ht[m][:, sl],
                    scalar1=s1[m], scalar2=t1[m], op0=OP.mult, op1=OP.add)
        for mj in range(EC):
            ps = ps_big.tile([128, 1024], F32, tag="sc", name="psf1")
            for c0, cl in _bank_slices(o0, ln):
                csl = slice(o0 + c0, o0 + c0 + cl)
                for g in range(2):
                    nc.tensor.matmul(ps[:, c0:c0 + cl],
                                     f18[:, g, :, mj * 128:(mj + 1) * 128],
                                     ht8[g][:, :, csl],
                                     start=(g == 0), stop=(g == 1),
                                     perf_mode=DR)
            with nc.allow_low_precision(reason="fp8 FFN operands"):
                nc.scalar.activation(out=h18[mj // 2][:, mj % 2, sl],
                                     in_=ps[:, 0:ln], func=AF.Relu,
                                     bias=bias["f1"][:, mj:mj + 1],
                                     scale=1.0)
        # the mid slice's deferred stats ride the last slice's FFN1 phase,
        # off the slice-to-slice dependency chain
        for t in deferred_stats:
            t()
        deferred_stats = []
        for m in range(EC):
            ps = ps_big.tile([128, 1024], F32, tag="sc", name="psf2")
            for c0, cl in _bank_slices(o0, ln):
                csl = slice(o0 + c0, o0 + c0 + cl)
                for g in range(2):
                    nc.tensor.matmul(ps[:, c0:c0 + cl],
                                     f28[:, g, :, m * 128:(m + 1) * 128],
                                     h18[g][:, :, csl],
                                     start=(g == 0), stop=(g == 1),
                                     perf_mode=DR)
            # ho = f2@h1 + BN1(ht); the f2 bias and BN1 shift are per-channel
            # constants erased by BN2, so only the s1 scale survives here
            nc.vector.scalar_tensor_tensor(
                out=ho[m][:, sl], in0=ht[m][:, sl], scalar=s1[m],
                in1=ps[:, 0:ln], op0=OP.mult, op1=OP.add)
            for gi, gg in enumerate(grps):
                def emit_stats(m=m, gg=gg, a=o0 + gi * 512, b=o0 + (gi + 1) * 512):
                    nc.vector.bn_stats(out=st2_tiles[m][:, gg, :],
                                       in_=ho[m][:, a:b])
                if si == 1:
                    deferred_stats.append(emit_stats)
                else:
                    emit_stats()

    # ---------- BN2 + bf16 output (pipelined per 512-token slice) ----------
    s2, t2 = _bn_params(nc, small, dram, st2_tiles, bias["bn2_g"],
                        bias["bn2_b"], "bn2", for_timing)
    yb = [const.tile([128, TOK], BF16, tag=f"xb{m}", name=f"yb{m}")
          for m in range(EC)]
    for s in range(4):
        sl = slice(s * 512, (s + 1) * 512)
        for m in range(EC):
            with nc.allow_low_precision(reason="bf16 output"):
                if m % 2 == 0:
                    nc.vector.tensor_scalar(out=yb[m][:, sl], in0=ho[m][:, sl],
                                            scalar1=s2[m], scalar2=t2[m],
                                            op0=OP.mult, op1=OP.add)
                else:
                    nc.scalar.activation(out=yb[m][:, sl], in_=ho[m][:, sl],
                                         func=AF.Identity, bias=t2[m],
                                         scale=s2[m])
            nc.sync.dma_start(out=yT_d.ap()[m * 128:(m + 1) * 128, sl],
                              in_=yb[m][:, sl])


def _bn_stats_tiles(small, name, groups):
    return [small.tile([128, groups, 6], F32, tag=f"st_{name}{m}",
                       name=f"st_{name}{m}", bufs=1) for m in range(EC)]


def _bn_params(nc, small, dram, sts, g_sb, b_sb, name, for_timing=False):
    """Per-channel scale/shift for training-mode BN over all B*N tokens:
    local sums (bn_stats emitted earlier into sts) -> 8-core AllReduce ->
    mu/var -> ACT rsqrt + one Newton step.
    Returns ([EC] scale APs, [EC] shift APs), each [128, 1]."""
    ccin = dram.tile([128, 2 * EC], F32, tag=f"cci_{name}", name=f"cci_{name}")
    ccout = dram.tile([128, 2 * EC], F32, tag=f"cco_{name}", name=f"cco_{name}")
    su = small.tile([128, 2 * EC], F32, tag=f"su_{name}", name=f"su_{name}")
    mva = small.tile([128, EC, 2], F32, tag=f"mv_{name}", name=f"mv_{name}",
                     bufs=1)
    for m in range(EC):
        nc.vector.bn_aggr(out=mva[:, m, :], in_=sts[m][:])
    # su0 = sum(h) = mean * TOK ; su1 = sum(h^2) = (var + mean^2) * TOK
    suv = su[:].rearrange("p (c two) -> p c two", two=2)
    t = small.tile([128, EC], F32, tag=f"tmp_{name}", name=f"tmp_{name}",
                   bufs=1)
    nc.vector.tensor_scalar(out=suv[:, :, 0], in0=mva[:, :, 0],
                            scalar1=float(TOK), scalar2=None, op0=OP.mult)
    nc.vector.tensor_tensor(out=t[:], in0=mva[:, :, 0], in1=mva[:, :, 0],
                            op=OP.mult)
    nc.vector.tensor_tensor(out=t[:], in0=t[:], in1=mva[:, :, 1], op=OP.add)
    nc.vector.tensor_scalar(out=suv[:, :, 1], in0=t[:], scalar1=float(TOK),
                            scalar2=None, op0=OP.mult)
    nc.sync.dma_start(out=ccin[:], in_=su[:])
    if for_timing:
        # TimelineSim cannot model collectives; substitute a same-shape copy
        nc.gpsimd.dma_start(out=ccout[:], in_=ccin[:])
    else:
        nc.gpsimd.collective_compute(
            "AllReduce", OP.add, replica_groups=[list(range(N_CORES))],
            ins=[ccin.opt()], outs=[ccout.opt()])
    scales, shifts = [], []
    gsa = small.tile([128, 2 * EC], F32, tag=f"gs_{name}", name=f"gs_{name}")
    nc.sync.dma_start(out=gsa[:], in_=ccout[:])
    gv = gsa[:].rearrange("p (c two) -> p c two", two=2)
    mu = small.tile([128, EC], F32, tag=f"mu_{name}", name=f"mu_{name}", bufs=1)
    var = small.tile([128, EC], F32, tag=f"var_{name}", name=f"var_{name}",
                     bufs=1)
    t2 = small.tile([128, EC], F32, tag=f"t2_{name}", name=f"t2_{name}", bufs=1)
    nc.vector.tensor_scalar(out=mu[:], in0=gv[:, :, 0],
                            scalar1=1.0 / N_GLOBAL, scalar2=None, op0=OP.mult)
    nc.vector.tensor_scalar(out=t2[:], in0=gv[:, :, 1],
                            scalar1=1.0 / N_GLOBAL, scalar2=None, op0=OP.mult)
    nc.vector.tensor_tensor(out=var[:], in0=mu[:], in1=mu[:], op=OP.mult)
    nc.vector.tensor_tensor(out=var[:], in0=t2[:], in1=var[:], op=OP.subtract)
    # r = 1/sqrt(var + eps): ACT Sqrt + DVE reciprocal, then one Newton step
    # to wash out the sqrt table's loose ULP budget
    epst = small.tile([128, 1], F32, tag=f"eps_{name}", name=f"eps_{name}",
                      bufs=1)
    nc.vector.memset(epst[:], EPS)
    sq = small.tile([128, EC], F32, tag=f"sq_{name}", name=f"sq_{name}", bufs=1)
    nc.scalar.activation(out=sq[:], in_=var[:], func=AF.Sqrt, bias=epst[:],
                         scale=1.0)
    r0 = small.tile([128, EC], F32, tag=f"r0_{name}", name=f"r0_{name}", bufs=1)
    nc.vector.reciprocal(out=r0[:], in_=sq[:])
    av_ = small.tile([128, EC], F32, tag=f"a_{name}", name=f"a_{name}", bufs=1)
    nc.vector.tensor_scalar(out=av_[:], in0=var[:], scalar1=EPS, scalar2=None,
                            op0=OP.add)
    nt = small.tile([128, EC], F32, tag=f"nt_{name}", name=f"nt_{name}", bufs=1)
    nc.vector.tensor_tensor(out=nt[:], in0=r0[:], in1=r0[:], op=OP.mult)
    nc.vector.tensor_tensor(out=nt[:], in0=nt[:], in1=av_[:], op=OP.mult)
    nc.vector.tensor_scalar(out=nt[:], in0=nt[:], scalar1=-0.5, scalar2=1.5,
                            op0=OP.mult, op1=OP.add)
    r = small.tile([128, EC], F32, tag=f"r_{name}", name=f"r_{name}", bufs=1)
    nc.vector.tensor_tensor(out=r[:], in0=r0[:], in1=nt[:], op=OP.mult)
    s_all = small.tile([128, EC], F32, tag=f"s_{name}", name=f"s_{name}",
                       bufs=1)
    sh_all = small.tile([128, EC], F32, tag=f"sh_{name}", name=f"sh_{name}",
                        bufs=1)
    nc.vector.tensor_tensor(out=s_all[:], in0=r[:], in1=g_sb, op=OP.mult)
    nc.vector.tensor_tensor(out=sh_all[:], in0=mu[:], in1=s_all[:], op=OP.mult)
    nc.vector.tensor_tensor(out=sh_all[:], in0=b_sb, in1=sh_all[:],
                            op=OP.subtract)
    for m in range(EC):
        scales.append(s_all[:, m:m + 1])
        shifts.append(sh_all[:, m:m + 1])
    return scales, shifts


_NC_CACHE = None


def _get_nc():
    global _NC_CACHE
    if _NC_CACHE is None:
        _NC_CACHE = build()
    return _NC_CACHE


def make_in_maps(inputs):
    import ml_dtypes
    f8 = ml_dtypes.float8_e4m3
    bf = ml_dtypes.bfloat16

    shared = {}
    for n in W_NAMES:
        w = np.asarray(inputs[f"{n}_w"], np.float32)       # [E_out, E_in]
        # w8[p, g, i, j] = W[j, (2g+i)*128 + p]
        wt = w.T.reshape(2, 2, 128, E)                     # [g, i, p, j]
        shared[f"{n}_w8"] = np.ascontiguousarray(
            wt.transpose(2, 0, 1, 3)).astype(f8)
    bpk = np.empty((128, len(B_NAMES) * EC), dtype=np.float32)
    for i, n in enumerate(B_NAMES):
        vec = inputs[f"{n}_b"] if n in W_NAMES else inputs[n]
        bpk[:, i * EC:(i + 1) * EC] = np.asarray(vec).reshape(EC, 128).T
    shared["bpk"] = bpk
    sel2 = np.zeros((98, 128), dtype=np.float32)
    for p in range(4):
        sel2[32 * p, 0:64] = 1.0
        sel2[32 * p + 1, 64:128] = 1.0
    shared["sel2"] = sel2.astype(bf)
    shared["idb"] = np.eye(128, dtype=np.float32).astype(bf)
    shared["ones8"] = np.ones((128, 3 * 2 * H * 2),
                               dtype=np.float32).astype(f8)

    x = np.asarray(inputs["x"], dtype=np.float32)
    in_maps = []
    for i in range(N_CORES):
        xc = x[BL * i:BL * (i + 1)]                        # [BL, N, E]
        xT = np.ascontiguousarray(xc.transpose(2, 0, 1).reshape(E, TOK))
        x8 = np.ascontiguousarray(
            xT.reshape(2, 2, 128, TOK).transpose(2, 0, 1, 3)).astype(f8)
        in_maps.append({"x8": x8, "xb": xT.astype(bf), **shared})
    return in_maps


def assemble_output(results):
    y = np.empty((B, N, E), dtype=np.float32)
    for i in range(N_CORES):
        yT = np.asarray(results[i]["yT"]).astype(np.float32)   # [E, TOK]
        y[BL * i:BL * (i + 1)] = yT.reshape(E, BL, N).transpose(1, 2, 0)
    return y


def kernel(**inputs):
    nc = _get_nc()
    in_maps = make_in_maps(inputs)
    res = run_bass_kernel_spmd(nc, in_maps, core_ids=list(range(N_CORES)))
    return assemble_output(res.results)


if __name__ == "__main__":
    nc = build()
    print("build ok")


# revision 75
# speedup vs baseline: 1.0076x; 1.0076x over previous
"""Trainium2 Bass kernel for nn_EncoderBlock (dual self-attention + BN + FFN + BN).

Sharding: data-parallel over batch (16 batches -> 2 per core on 8 cores).
Device layout: activations transposed (channels E on partitions, tokens on free
dim) so BatchNorm stats are free-dim reductions. Attention computes transposed
scores sT[h] = k_h @ q_h.T so softmax needs no on-device transposes; a ones
column appended to V produces softmax denominators inside the AV matmul; the
per-query reciprocal denominators are broadcast across partitions with a tiny
K=2 matmul.

Precision plan (rel-err budget 2e-2, lands ~1e-2):
- All weight matmuls (QKV/O projections, FFN) run in fp8e4m3 with DoubleRow
  perf mode: two 128-deep contraction tiles per instruction at 0.5 PE
  cycles/row. Weights are pre-quantized on the host in [128, 2, E] k-tile-pair
  layout; activations are quantized on the fly during PSUM evacuation.
- Scores stay f32r (contraction is dk=64: DoubleRow's k-tile pairing cannot
  apply, and fp8 without DoubleRow has no rate advantage).
- exp(scores) is written directly as fp8 and consumed by a DoubleRow AV
  matmul against fp8 V (ones column included for denominators).
- The residual x is added inside the output-projection PSUM group via a
  bf16 identity matmul (x is loaded bf16 only; f32 x is never needed).
- Output is written bf16 and upcast on the host.

Algebraic eliminations (exact): K-projection bias is softmax-invariant
(q+bq)@(k+bk) == (q+bq)@k + per-query const; V/O/FFN2 biases and the BN1
shift in the FFN2 residual are per-channel constants which the following
BatchNorm subtracts out exactly. Only the Q bias, FFN1 bias (inside relu),
and the BN affine outputs are applied.

BatchNorm batch stats use a 4KB AllReduce across the 8 cores (twice); in
for_timing builds the collective is replaced by a same-shape DRAM copy
(TimelineSim cannot model collectives; the real kernel runs the AllReduce).
"""

import numpy as np
import concourse.bass as bass
import concourse.bacc as bacc
import concourse.tile as tile
from concourse import mybir
from concourse.bass_utils import run_bass_kernel_spmd

dt = mybir.dt
F32 = dt.float32
F32R = dt.float32r
BF16 = dt.bfloat16
F8 = dt.float8e4
AF = mybir.ActivationFunctionType
OP = mybir.AluOpType
DR = mybir.MatmulPerfMode.DoubleRow

N_CORES = 8
B, N, E, H, DK = 16, 1024, 512, 8, 64
NR, NT = 256, 768          # robot / task sequence lengths
BL = B // N_CORES          # local batches per core
TOK = BL * N               # local tokens per core
EC = E // 128              # channel chunks of 128
N_GLOBAL = B * N           # BN stat count
EPS = 1e-5

W_NAMES = ["rq", "rk", "rv", "ro", "tq", "tk", "tv", "to", "f1", "f2"]
B_NAMES = ["rq", "tq", "f1", "bn1_g", "bn1_b", "bn2_g", "bn2_b"]


def _bank_slices(base, length):
    """Split [base, base+length) into pieces (<=512) that never cross a
    512-col PSUM bank boundary. base/length multiples of 256."""
    out = []
    cur = base
    end = base + length
    while cur < end:
        nb = (cur // 512 + 1) * 512
        fl = min(end, nb) - cur
        out.append((cur - base, fl))
        cur += fl
    return out


def build(for_timing=False):
    nc = bacc.Bacc("TRN2", target_bir_lowering=False, debug=False,
                   num_devices=N_CORES)

    x8_d = nc.dram_tensor("x8", [128, 2, 2, TOK], F8, kind="ExternalInput")
    xb_d = nc.dram_tensor("xb", [E, TOK], BF16, kind="ExternalInput")
    w_d = {n: nc.dram_tensor(f"{n}_w8", [128, 2, 2, E], F8,
                             kind="ExternalInput")
           for n in W_NAMES}
    bpk_d = nc.dram_tensor("bpk", [128, len(B_NAMES) * EC], F32,
                           kind="ExternalInput")
    sel2_d = nc.dram_tensor("sel2", [98, 128], BF16,
                            kind="ExternalInput")
    id_d = nc.dram_tensor("idb", [128, 128], BF16, kind="ExternalInput")
    ones_d = nc.dram_tensor("ones8", [128, 3 * 2 * H * 2], F8,
                             kind="ExternalInput")
    yT_d = nc.dram_tensor("yT", [E, TOK], BF16, kind="ExternalOutput")

    from contextlib import ExitStack
    with tile.TileContext(nc) as tc, ExitStack() as es:
        const = es.enter_context(tc.tile_pool(name="const", bufs=1))
        wpool = es.enter_context(tc.tile_pool(name="w", bufs=1))
        act = es.enter_context(tc.tile_pool(name="act", bufs=1))
        attn = es.enter_context(tc.tile_pool(name="attn", bufs=1))
        expp = es.enter_context(tc.tile_pool(name="expp", bufs=2))
        small = es.enter_context(tc.tile_pool(name="small", bufs=2))
        dram = es.enter_context(tc.tile_pool(name="dram", bufs=1, space="DRAM"))
        ps_big = es.enter_context(tc.tile_pool(name="ps_big", bufs=2, space="PSUM"))
        ps_av = es.enter_context(tc.tile_pool(name="ps_av", bufs=2, space="PSUM"))
        _body(nc, const, wpool, act, attn, expp, small, dram, ps_big, ps_av,
              x8_d, xb_d, w_d, bpk_d, sel2_d, id_d, ones_d, yT_d, for_timing)
    nc.finalize()
    return nc


def _load_w8(nc, wpool, w_dram, tag, queue=None):
    """fp8 weight matrix as one [128, 2, 2, E] tile (g-major k-tile pairs),
    loaded in a single DMA."""
    t = wpool.tile([128, 2, 2, E], F8, tag=tag, name=tag)
    (queue or nc.sync).dma_start(out=t[:], in_=w_dram.ap())
    return t


class _Ctx:
    pass


def _body(nc, const, wpool, act, attn, expp, small, dram, ps_big, ps_av,
          x8_d, xb_d, w_d, bpk_d, sel2_d, id_d, ones_d, yT_d, for_timing):
    # ---------- constants / inputs resident in SBUF ----------
    # DMA emission order == HWDGE issue order: the first projection needs
    # bpk + wq + x8 robot-b0 columns; everything else trickles in after
    bpk = const.tile([128, len(B_NAMES) * EC], F32, tag="bpk", name="bpk")
    nc.sync.dma_start(out=bpk[:], in_=bpk_d.ap())
    bias = {n: bpk[:, i * EC:(i + 1) * EC] for i, n in enumerate(B_NAMES)}

    # latency-critical robot-phase loads on the SP HWDGE queue (the fused
    # robot part runs first; the task part follows)
    w8 = {"rq": _load_w8(nc, wpool, w_d["rq"], "wrq")}
    x8 = [const.tile([128, 2, TOK], F8, tag=f"x8_{g}", name=f"x8_{g}")
          for g in range(2)]
    for g in range(2):
        nc.sync.dma_start(
            out=x8[g][:].rearrange("p i (b n) -> p i b n", b=2)[:, :, :, :NR],
            in_=x8_d.ap()[:, g, :, :].rearrange(
                "p i (b n) -> p i b n", b=2)[:, :, :, :NR])
    w8["rk"] = _load_w8(nc, wpool, w_d["rk"], "wrk")
    w8["rv"] = _load_w8(nc, wpool, w_d["rv"], "wrv")
    sel2 = const.tile([98, 128], BF16, tag="sel2", name="sel2")
    nc.sync.dma_start(out=sel2[:], in_=sel2_d.ap())
    idb = const.tile([128, 128], BF16, tag="idb", name="idb")
    nc.sync.dma_start(out=idb[:], in_=id_d.ap())
    w8["ro"] = _load_w8(nc, wpool, w_d["ro"], "wro")
    xb = [const.tile([128, TOK], BF16, tag=f"xb{m}", name=f"xb{m}")
          for m in range(EC)]
    for m in range(EC):
        nc.sync.dma_start(
            out=xb[m][:].rearrange("p (b n) -> p b n", b=2)[:, :, :NR],
            in_=xb_d.ap()[m * 128:(m + 1) * 128, :].rearrange(
                "p (b n) -> p b n", b=2)[:, :, :NR])

    # persistent fp8 V tiles, one per batch parity; the ones column
    # (softmax denominator trick) is initialized once, first in the SWDGE
    # queue so robot AV is never blocked on it
    v_all = [attn.tile([128, 3, 2, H, DK + 4], F8, tag=f"v_{par}",
                       name=f"v_{par}") for par in range(2)]
    for par in range(2):
        nc.gpsimd.dma_start(
            out=v_all[par][:, :, :, :, DK:DK + 2],
            in_=ones_d.ap().rearrange("p (t i h one) -> p t i h one",
                                      t=3, i=2, one=2))

    # bulk task/FFN loads go through the Pool SWDGE queue so they never sit
    # in front of the latency-critical robot transfers on SP, and so their
    # issue overhead stays off the SP sequencer
    for g in range(2):
        nc.gpsimd.dma_start(
            out=x8[g][:].rearrange("p i (b n) -> p i b n", b=2)[:, :, :, NR:],
            in_=x8_d.ap()[:, g, :, :].rearrange(
                "p i (b n) -> p i b n", b=2)[:, :, :, NR:])
    for nm in ["tq", "tk", "tv", "to"]:
        w8[nm] = _load_w8(nc, wpool, w_d[nm], f"w{nm}", queue=nc.gpsimd)
    for m in range(EC):
        nc.gpsimd.dma_start(
            out=xb[m][:].rearrange("p (b n) -> p b n", b=2)[:, :, NR:],
            in_=xb_d.ap()[m * 128:(m + 1) * 128, :].rearrange(
                "p (b n) -> p b n", b=2)[:, :, NR:])
    for nm in ["f1", "f2"]:
        w8[nm] = _load_w8(nc, wpool, w_d[nm], f"w{nm}", queue=nc.gpsimd)

    # prefetch the exp ACT table set while input DMAs are in flight
    warm = const.tile([1, 1], F32, tag="warm", name="warm")
    nc.vector.memset(warm[:], 0.0)
    nc.scalar.activation(out=warm[:], in_=warm[:], func=AF.Exp, scale=1.0)

    # h-tilde (pre-BN1 attention output) accumulated across parts/batches
    ht = [act.tile([128, TOK], F32, tag=f"ht{k}", name=f"ht{k}")
          for k in range(EC)]

    # ---------- attention (emission software-pipelined) ----------
    # The task part (Np=768) runs per batch; the robot part (Np=256) fuses
    # both batches into one block-diagonal 512-column stream (b*256+t) so
    # its latency chains are half as many and twice as wide.
    def _bview(t):
        return t.rearrange("p (b n) -> p b n", b=2)

    def proj_steps(P, b):
        """Allocate per-batch projection state and return (st, steps):
        emission thunks, one per channel chunk, each emitting the q/k
        projections for chunk m plus the scheduled V chunks."""
        st = _Ctx()
        st.tok0 = b * N + NR if not P.fused else None
        st.b = b
        st.qT = [attn.tile([128, NT], BF16, tag=f"qT{m}_{b}", name=f"qT{m}_{b}")
                 for m in range(EC)]
        st.kT = [attn.tile([128, NT], BF16, tag=f"kT{m}_{b}", name=f"kT{m}_{b}")
                 for m in range(EC)]
        st.ex = [None] * H
        if P.fused:
            st.zT = [attn.tile([128, 2 * NR], BF16, tag=f"zTr{p}",
                               name=f"zTr{p}") for p in range(4)]
            st.z8 = [attn.tile([128, 2, 2 * NR], F8, tag=f"z8r_{g}",
                               name=f"z8r_{g}") for g in range(2)]
            st.rows = small.tile([98, 2 * NR], BF16, tag="rowsr",
                                 name="rowsr", bufs=1)
            st.rinv = small.tile([98, 2 * NR], BF16, tag="rinvr",
                                 name="rinvr", bufs=1)
        else:
            st.zT = [attn.tile([128, NT], BF16, tag=f"zT{p}", name=f"zT{p}")
                     for p in range(4)]
            st.z8 = [attn.tile([128, 2, NT], F8, tag=f"z8_{g}_{b}",
                               name=f"z8_{g}_{b}") for g in range(2)]
            st.rows = small.tile([98, NT], BF16, tag="rows", name="rows",
                                 bufs=1)
            st.rinv = small.tile([98, NT], BF16, tag="rinv", name="rinv",
                                 bufs=1)

        def emit_qk(w_t, o_t, m, with_bias):
            # projection psums live in the av-tag slot: their DVE-paced
            # evacuations must never gate the score-psum rotation
            ps = ps_av.tile([128, NT], F32, tag="av", name="psq", bufs=1)
            wsl = w_t[:, :, :, m * 128:(m + 1) * 128]
            if P.fused:
                nc.tensor.matmul(
                    _bview(ps[:, 0:P.np]),
                    wsl[:, 0, :, :],
                    x8[0][:].rearrange("p i (b n) -> p i b n",
                                       b=2)[:, :, :, 0:NR],
                    start=True, stop=False, perf_mode=DR)
                nc.tensor.matmul(
                    _bview(ps[:, 0:P.np]),
                    wsl[:, 1, :, :],
                    x8[1][:].rearrange("p i (b n) -> p i b n",
                                       b=2)[:, :, :, 0:NR],
                    start=False, stop=True, perf_mode=DR)
            else:
                for off, fl in _bank_slices(0, P.np):
                    for g in range(2):
                        nc.tensor.matmul(
                            ps[:, off:off + fl], wsl[:, g, :, :],
                            x8[g][:, :, st.tok0 + off:st.tok0 + off + fl],
                            start=(g == 0), stop=(g == 1), perf_mode=DR)
            with nc.allow_low_precision(reason="bf16 attn operands"):
                if with_bias and P.fused:
                    # robot-phase Q evacs ride the otherwise-idle Act engine
                    nc.scalar.activation(
                        out=o_t[m][:, 0:P.np], in_=ps[:, 0:P.np],
                        func=AF.Identity, bias=bias[P.wn[0]][:, m:m + 1],
                        scale=1.0)
                elif with_bias:
                    nc.vector.tensor_scalar(
                        out=o_t[m][:, 0:P.np], in0=ps[:, 0:P.np],
                        scalar1=bias[P.wn[0]][:, m:m + 1], scalar2=None,
                        op0=OP.add)
                else:
                    # K bias is softmax-invariant: plain evacuation
                    nc.vector.tensor_copy(out=o_t[m][:, 0:P.np],
                                          in_=ps[:, 0:P.np])

        def emit_v(t):
            # V bias is erased by BN1: plain quantizing evacuation
            # (GPSIMD cannot read PSUM, so this stays on DVE)
            if P.fused:
                vb, tt = t // 2, t % 2
                tok = vb * N + tt * 128
            else:
                vb, tt = b, None
                tok = st.tok0 + t * 128
            # robot V psums ride the sc rotation: their evacuations feed the
            # deferred AZ units, not the latency-critical q/k->score chain
            if P.fused:
                ps = ps_big.tile([128, E], F32, tag="sc", name="psv")
            else:
                ps = ps_av.tile([128, E], F32, tag="av", name="psv", bufs=1)
            for g in range(2):
                nc.tensor.matmul(
                    ps[:], x8[g][:, :, tok:tok + 128],
                    P.wv[:, g, :, :], start=(g == 0), stop=(g == 1),
                    perf_mode=DR)
            slot = (0, tt) if P.fused else (t // 2, t % 2)
            with nc.allow_low_precision(reason="fp8 AV operands"):
                nc.vector.tensor_copy(
                    out=v_all[vb][:, slot[0], slot[1], :, 0:DK],
                    in_=ps[:].rearrange("p (h d) -> p h d", h=H))

        v_sched = ([[0, 1], [2, 3], [4], [5]] if not P.fused
                   else [[0, 1], [2, 3], [], []])

        def step(m):
            emit_qk(P.wq, st.qT, m, True)
            emit_qk(P.wk, st.kT, m, False)
            for t in v_sched[m]:
                emit_v(t)

        return st, [lambda m=m: step(m) for m in range(EC)]

    def heads_se(P, st, h):
        """Scores + fp8 exp for one head."""
        Np = P.np
        qh = st.qT[h // 2][(h % 2) * 64:(h % 2) * 64 + 64, 0:Np]
        kh = st.kT[h // 2][(h % 2) * 64:(h % 2) * 64 + 64, 0:Np]
        st.ex[h] = []
        if P.fused:
            sc = ps_big.tile([128, 2, NT], F32, tag="sc", name="sc")
            for vb in range(2):
                for i in range(2):
                    nc.tensor.matmul(
                        sc[:, i, vb * 256:(vb + 1) * 256],
                        kh[:, vb * 256 + i * 128:vb * 256 + (i + 1) * 128],
                        qh[:, vb * 256:(vb + 1) * 256],
                        start=True, stop=True)
            ex = expp.tile([128, 2, 2 * NR], F8, tag="exr", name="exr",
                           bufs=8)
            with nc.allow_low_precision(reason="fp8 AV operands"):
                nc.scalar.activation(out=ex[:, :, 0:Np], in_=sc[:, :, 0:Np],
                                     func=AF.Exp, scale=0.125)
            st.ex[h].append(ex)
            return
        for grp in range(P.nk // 2):
            sc = ps_big.tile([128, 2, NT], F32, tag="sc", name="sc")
            for j2 in range(2):
                kc = 2 * grp + j2
                for off, fl in _bank_slices(j2 * NT, Np):
                    nc.tensor.matmul(sc[:, j2, off:off + fl],
                                     kh[:, kc * 128:(kc + 1) * 128],
                                     qh[:, off:off + fl],
                                     start=True, stop=True)
            ex = expp.tile([128, 2, NT], F8, tag="exp", name="exp", bufs=10)
            with nc.allow_low_precision(reason="fp8 AV operands"):
                nc.scalar.activation(out=ex[:, :, 0:Np], in_=sc[:, :, 0:Np],
                                     func=AF.Exp, scale=0.125)
            st.ex[h].append(ex)

    def heads_az(P, st, h):
        """DoubleRow AV (+denominator row) for one head; psum evacuated
        partition-aligned then remapped into pair tiles via DMA."""
        Np = P.np
        pair, j = h // 2, h % 2
        zu = ps_av.tile([66, NT], F32, tag="av", name="av", bufs=1)
        if P.fused:
            for vb in range(2):
                nc.tensor.matmul(
                    zu[:, vb * 256:(vb + 1) * 256],
                    v_all[vb][:, 0, :, h, 0:DK + 2],
                    st.ex[h][0][:, :, vb * 256:(vb + 1) * 256],
                    start=True, stop=True, perf_mode=DR)
        else:
            for grp in range(P.nk // 2):
                for off, fl in _bank_slices(0, Np):
                    nc.tensor.matmul(
                        zu[:, off:off + fl],
                        v_all[st.b][:, grp, :, h, 0:DK + 2],
                        st.ex[h][grp][:, :, off:off + fl],
                        start=(grp == 0), stop=(grp == P.nk // 2 - 1),
                        perf_mode=DR)
        st.ex[h] = None
        zst = expp.tile([65, NT], BF16, tag="zst", name="zst", bufs=3)
        zu = zu[0:65, :]
        with nc.allow_low_precision(reason="z is fp8-quantized downstream"):
            if P.fused:
                nc.scalar.activation(out=zst[:, 0:Np], in_=zu[:, 0:Np],
                                     func=AF.Identity, scale=1.0)
            else:
                nc.vector.tensor_copy(out=zst[:, 0:Np], in_=zu[:, 0:Np])
        nc.sync.dma_start(out=st.zT[pair][j * 64:(j + 1) * 64, 0:Np],
                          in_=zst[0:64, 0:Np])
        nc.sync.dma_start(
            out=st.rows[32 * pair + j:32 * pair + j + 1, 0:Np],
            in_=zst[64:65, 0:Np])

    def denom_pair(P, st, pair):
        """Reciprocal of the pair's denominators (deferred a step so the DVE
        queue never stalls on the rowsum DMA), broadcast across partitions
        (K=2 matmul) and folded into z while quantizing to fp8."""
        Np = P.np
        with nc.allow_low_precision(reason="z is fp8-quantized downstream"):
            nc.vector.reciprocal(
                out=st.rinv[32 * pair:32 * pair + 2, 0:Np],
                in_=st.rows[32 * pair:32 * pair + 2, 0:Np])
        rinv = st.rinv[32 * pair:32 * pair + 2, 0:Np]
        rep = ps_av.tile([128, NT], F32, tag="av", name="rep", bufs=1)
        tp = (96, 0) if pair == 3 else None
        for off, fl in _bank_slices(0, Np):
            nc.tensor.matmul(rep[:, off:off + fl],
                             sel2[32 * pair:32 * pair + 2, :],
                             rinv[:, off:off + fl], start=True, stop=True,
                             tile_position=tp)
        with nc.allow_low_precision(reason="fp8 outproj operands"):
            nc.vector.tensor_tensor(out=st.z8[pair // 2][:, pair % 2, 0:Np],
                                    in0=st.zT[pair][:, 0:Np],
                                    in1=rep[:, 0:Np], op=OP.mult)

    def outproj_m(P, st, m, tail=False, on_sc=False):
        """One channel chunk of the DoubleRow output projection of the
        normalized fp8 z, with the residual added via a bf16 identity
        matmul, evacuated into ht with BN1 stats for the newly completed
        256-col slices. `tail` (very last batch) runs on the free score
        psum and evacuates on Act so the BN1 chain is as short as
        possible."""
        Np = P.np
        if tail or on_sc:
            ps = ps_big.tile([128, NT], F32, tag="sc", name="pso")
        else:
            ps = ps_av.tile([128, NT], F32, tag="av", name="pso", bufs=1)
        for off, fl in _bank_slices(0, Np):
            for g in range(2):
                nc.tensor.matmul(ps[:, off:off + fl],
                                 P.wo[:, g, :, m * 128:(m + 1) * 128],
                                 st.z8[g][:, :, off:off + fl],
                                 start=(g == 0), stop=False, perf_mode=DR)
            # residual: += I128 @ x (bf16); O bias is erased by BN1
            if P.fused:
                nc.tensor.matmul(_bview(ps[:, 0:Np]), idb[:],
                                 _bview(xb[m][:])[:, :, 0:NR],
                                 start=False, stop=True)
            else:
                nc.tensor.matmul(ps[:, off:off + fl], idb[:],
                                 xb[m][:, st.tok0 + off:st.tok0 + off + fl],
                                 start=False, stop=True)
            if P.fused:
                break
        if P.fused:
            dst = _bview(xb[m][:])[:, :, 0:NR]
            dst = _bview(ht[m][:])[:, :, 0:NR]
            src_ = _bview(ps[:, 0:Np])
        else:
            dst = ht[m][:, st.tok0:st.tok0 + Np]
            src_ = ps[:, 0:Np]
        if tail:
            nc.scalar.activation(out=dst, in_=src_, func=AF.Identity,
                                 scale=1.0)
        else:
            nc.vector.tensor_copy(out=dst, in_=src_)
        # emit BN1 stats for the 256-col slices this part/batch completes,
        # so only the last slice's stats sit on the BN1 critical path
        for c in P.st_slices[st.b]:
            nc.vector.bn_stats(out=st1_tiles[m][:, c, :],
                               in_=ht[m][:, c * 256:(c + 1) * 256])

    from collections import deque
    backlog = deque()

    def pump():
        if backlog:
            backlog.popleft()()

    def attention_batch(P, st, psteps_next, pumps=2, se_next=None):
        """One batch's AZ stream with denominators folded in; each step also
        emits one deferred unit: the next batch's projections first, then
        whatever is in the backlog (the previous batch's output
        projection)."""
        for h in range(H):
            heads_az(P, st, h)
            if h % 2 == 0 and h > 0:
                denom_pair(P, st, h // 2 - 1)
            if h + 3 < H:
                heads_se(P, st, h + 3)
            elif se_next:
                se_next.popleft()()
            if psteps_next:
                psteps_next.popleft()()
            else:
                for _ in range(pumps):
                    pump()
        denom_pair(P, st, 3)

    st1_tiles = _bn_stats_tiles(small, "bn1", 8)
    st2_tiles = _bn_stats_tiles(small, "bn2", 4)
    f18, f28 = w8["f1"], w8["f2"]

    # ----- robot part first: both batches fused block-diagonally; its
    # scores/exps run up front and everything downstream of the exps is
    # deferred into the task stream via the backlog -----
    R = _Ctx()
    R.part, R.fused = 0, True
    R.wn = ["rq", "rk", "rv", "ro"]
    R.np, R.nk = 2 * NR, 4
    R.st_slices = ([0, 4],)
    R.wq, R.wk, R.wv, R.wo = w8["rq"], w8["rk"], w8["rv"], w8["ro"]

    str_, pr = proj_steps(R, 0)
    for m in range(EC):
        pr[m]()
        heads_se(R, str_, m)
    for h in range(EC, H):
        heads_se(R, str_, h)
    rob = [lambda h=h: heads_az(R, str_, h) for h in range(H)]
    rob[3:3] = [lambda: denom_pair(R, str_, 0)]
    rob[6:6] = [lambda: denom_pair(R, str_, 1)]
    rob[9:9] = [lambda: denom_pair(R, str_, 2)]
    rob.append(lambda: denom_pair(R, str_, 3))
    rob += [lambda m=m: outproj_m(R, str_, m, on_sc=True)
            for m in range(EC)]
    backlog.extend(rob)

    # ----- task part: two batches, software-pipelined -----
    P = _Ctx()
    P.part, P.fused = 1, False
    P.wn = ["tq", "tk", "tv", "to"]
    P.np, P.nk = NT, 6
    P.st_slices = ([1, 2, 3], [5, 6, 7])
    P.wq, P.wk, P.wv, P.wo = w8["tq"], w8["tk"], w8["tv"], w8["to"]

    st0, p0 = proj_steps(P, 0)
    st1, p1 = proj_steps(P, 1)
    p0[0]()
    heads_se(P, st0, 0)
    pump()
    p0[1]()
    heads_se(P, st0, 1)
    pump()
    p0[2]()
    heads_se(P, st0, 2)
    pump()
    p0[3]()
    pump()
    attention_batch(P, st0, deque(p1), pumps=1,
                    se_next=deque([lambda h=h: heads_se(P, st1, h)
                                   for h in range(3)]))
    attention_batch(P, st1, None, pumps=1)
    while backlog:
        pump()
    # all exps done: swap the ACT table set to sqrt (the set also holds
    # relu/identity for the FFN and BN2 phases)
    warm2 = const.tile([1, 1], F32, tag="warm", name="warm2")
    nc.vector.memset(warm2[:], 1.0)
    nc.scalar.activation(out=warm2[:], in_=warm2[:], func=AF.Sqrt, scale=1.0)
    # both task batches' output projections as one wide tail: PE back-to-back
    # on the now-free score psum, Act evacuations in parallel with DVE stats
    for m in range(EC):
        outproj_m(P, st0, m, tail=True)
    for m in range(EC):
        outproj_m(P, st1, m, tail=True)

    # ---------- BN1 ----------
    s1, t1 = _bn_params(nc, small, dram, st1_tiles, bias["bn1_g"],
                        bias["bn1_b"], "bn1", for_timing)

    # ---------- FFN (token-slice pipelined; BN1 folded into the fp8
    # quantization of ht and into the FFN2 epilogue) ----------
    ht8 = [act.tile([128, 2, TOK], F8, tag=f"ht8_{g}", name=f"ht8_{g}")
           for g in range(2)]
    h18 = [act.tile([128, 2, TOK], F8, tag=f"h18_{g}", name=f"h18_{g}")
           for g in range(2)]
    ho = [act.tile([128, TOK], F32, tag=f"ho{k}", name=f"ho{k}")
          for k in range(EC)]
    FSL = [(0, 512, [0]), (512, 1024, [1, 2]), (1536, 512, [3])]
    deferred_stats = []
    for si, (o0, ln, grps) in enumerate(FSL):
        sl = slice(o0, o0 + ln)
        # quantize BN1(ht) for this token slice (first slice on the
        # just-idle DVE so FFN1 starts immediately; rest on gpsimd)
        with nc.allow_low_precision(reason="fp8 FFN operands"):
            for m in range(EC):
                eng = nc.vector if si == 0 else nc.gpsimd
                eng.tensor_scalar(
                    out=ht8[m // 2][:, m % 2, sl], in0=ht[m][:, sl],
                    scalar1=s1[m], scalar2=t1[m], op0=OP.mult, op1=OP.add)
        for mj in range(EC):
            ps = ps_big.tile([128, 1024], F32, tag="sc", name="psf1")
            for c0, cl in _bank_slices(o0, ln):
                csl = slice(o0 + c0, o0 + c0 + cl)
                for g in range(2):
                    nc.tensor.matmul(ps[:, c0:c0 + cl],
                                     f18[:, g, :, mj * 128:(mj + 1) * 128],
                                     ht8[g][:, :, csl],
                                     start=(g == 0), stop=(g == 1),
                                     perf_mode=DR)
            with nc.allow_low_precision(reason="fp8 FFN operands"):
                nc.scalar.activation(out=h18[mj // 2][:, mj % 2, sl],
                                     in_=ps[:, 0:ln], func=AF.Relu,
                                     bias=bias["f1"][:, mj:mj + 1],
                                     scale=1.0)
        # the mid slice's deferred stats ride the last slice's FFN1 phase,
        # off the slice-to-slice dependency chain
        for t in deferred_stats:
            t()
        deferred_stats = []
        for m in range(EC):
            ps = ps_big.tile([128, 1024], F32, tag="sc", name="psf2")
            for c0, cl in _bank_slices(o0, ln):
                csl = slice(o0 + c0, o0 + c0 + cl)
                for g in range(2):
                    nc.tensor.matmul(ps[:, c0:c0 + cl],
                                     f28[:, g, :, m * 128:(m + 1) * 128],
                                     h18[g][:, :, csl],
                                     start=(g == 0), stop=(g == 1),
                                     perf_mode=DR)
            # ho = f2@h1 + BN1(ht); the f2 bias and BN1 shift are per-channel
            # constants erased by BN2, so only the s1 scale survives here
            nc.vector.scalar_tensor_tensor(
                out=ho[m][:, sl], in0=ht[m][:, sl], scalar=s1[m],
                in1=ps[:, 0:ln], op0=OP.mult, op1=OP.add)
            for gi, gg in enumerate(grps):
                def emit_stats(m=m, gg=gg, a=o0 + gi * 512, b=o0 + (gi + 1) * 512):
                    nc.vector.bn_stats(out=st2_tiles[m][:, gg, :],
                                       in_=ho[m][:, a:b])
                if si == 1:
                    deferred_stats.append(emit_stats)
                else:
                    emit_stats()

    # ---------- BN2 + bf16 output (pipelined per 512-token slice) ----------
    s2, t2 = _bn_params(nc, small, dram, st2_tiles, bias["bn2_g"],
                        bias["bn2_b"], "bn2", for_timing)
    yb = [const.tile([128, TOK], BF16, tag=f"xb{m}", name=f"yb{m}")
          for m in range(EC)]
    for s in range(4):
        sl = slice(s * 512, (s + 1) * 512)
        for m in range(EC):
            with nc.allow_low_precision(reason="bf16 output"):
                if m % 2 == 0:
                    nc.vector.tensor_scalar(out=yb[m][:, sl], in0=ho[m][:, sl],
                                            scalar1=s2[m], scalar2=t2[m],
                                            op0=OP.mult, op1=OP.add)
                else:
                    nc.scalar.activation(out=yb[m][:, sl], in_=ho[m][:, sl],
                                         func=AF.Identity, bias=t2[m],
                                         scale=s2[m])
            nc.sync.dma_start(out=yT_d.ap()[m * 128:(m + 1) * 128, sl],
                              in_=yb[m][:, sl])


def _bn_stats_tiles(small, name, groups):
    return [small.tile([128, groups, 6], F32, tag=f"st_{name}{m}",
                       name=f"st_{name}{m}", bufs=1) for m in range(EC)]


def _bn_params(nc, small, dram, sts, g_sb, b_sb, name, for_timing=False):
    """Per-channel scale/shift for training-mode BN over all B*N tokens:
    local sums (bn_stats emitted earlier into sts) -> 8-core AllReduce ->
    mu/var -> ACT rsqrt + one Newton step.
    Returns ([EC] scale APs, [EC] shift APs), each [128, 1]."""
    ccin = dram.tile([128, 2 * EC], F32, tag=f"cci_{name}", name=f"cci_{name}")
    ccout = dram.tile([128, 2 * EC], F32, tag=f"cco_{name}", name=f"cco_{name}")
    su = small.tile([128, 2 * EC], F32, tag=f"su_{name}", name=f"su_{name}")
    mva = small.tile([128, EC, 2], F32, tag=f"mv_{name}", name=f"mv_{name}",
                     bufs=1)
    for m in range(EC):
        nc.vector.bn_aggr(out=mva[:, m, :], in_=sts[m][:])
    # su0 = sum(h) = mean * TOK ; su1 = sum(h^2) = (var + mean^2) * TOK
    suv = su[:].rearrange("p (c two) -> p c two", two=2)
    t = small.tile([128, EC], F32, tag=f"tmp_{name}", name=f"tmp_{name}",
                   bufs=1)
    nc.vector.tensor_scalar(out=suv[:, :, 0], in0=mva[:, :, 0],
                            scalar1=float(TOK), scalar2=None, op0=OP.mult)
    nc.vector.tensor_tensor(out=t[:], in0=mva[:, :, 0], in1=mva[:, :, 0],
                            op=OP.mult)
    nc.vector.tensor_tensor(out=t[:], in0=t[:], in1=mva[:, :, 1], op=OP.add)
    nc.vector.tensor_scalar(out=suv[:, :, 1], in0=t[:], scalar1=float(TOK),
                            scalar2=None, op0=OP.mult)
    nc.sync.dma_start(out=ccin[:], in_=su[:])
    if for_timing:
        # TimelineSim cannot model collectives; substitute a same-shape copy
        nc.gpsimd.dma_start(out=ccout[:], in_=ccin[:])
    else:
        nc.gpsimd.collective_compute(
            "AllReduce", OP.add, replica_groups=[list(range(N_CORES))],
            ins=[ccin.opt()], outs=[ccout.opt()])
    scales, shifts = [], []
    gsa = small.tile([128, 2 * EC], F32, tag=f"gs_{name}", name=f"gs_{name}")
    nc.sync.dma_start(out=gsa[:], in_=ccout[:])
    gv = gsa[:].rearrange("p (c two) -> p c two", two=2)
    mu = small.tile([128, EC], F32, tag=f"mu_{name}", name=f"mu_{name}", bufs=1)
    var = small.tile([128, EC], F32, tag=f"var_{name}", name=f"var_{name}",
                     bufs=1)
    t2 = small.tile([128, EC], F32, tag=f"t2_{name}", name=f"t2_{name}", bufs=1)
    nc.vector.tensor_scalar(out=mu[:], in0=gv[:, :, 0],
                            scalar1=1.0 / N_GLOBAL, scalar2=None, op0=OP.mult)
    nc.vector.tensor_scalar(out=t2[:], in0=gv[:, :, 1],
                            scalar1=1.0 / N_GLOBAL, scalar2=None, op0=OP.mult)
    nc.vector.tensor_tensor(out=var[:], in0=mu[:], in1=mu[:], op=OP.mult)
    nc.vector.tensor_tensor(out=var[:], in0=t2[:], in1=var[:], op=OP.subtract)
    # r = 1/sqrt(var + eps): ACT Sqrt + DVE reciprocal, then one Newton step
    # to wash out the sqrt table's loose ULP budget
    epst = small.tile([128, 1], F32, tag=f"eps_{name}", name=f"eps_{name}",
                      bufs=1)
    nc.vector.memset(epst[:], EPS)
    sq = small.tile([128, EC], F32, tag=f"sq_{name}", name=f"sq_{name}", bufs=1)
    nc.scalar.activation(out=sq[:], in_=var[:], func=AF.Sqrt, bias=epst[:],
                         scale=1.0)
    r0 = small.tile([128, EC], F32, tag=f"r0_{name}", name=f"r0_{name}", bufs=1)
    nc.vector.reciprocal(out=r0[:], in_=sq[:])
    av_ = small.tile([128, EC], F32, tag=f"a_{name}", name=f"a_{name}", bufs=1)
    nc.vector.tensor_scalar(out=av_[:], in0=var[:], scalar1=EPS, scalar2=None,
                            op0=OP.add)
    nt = small.tile([128, EC], F32, tag=f"nt_{name}", name=f"nt_{name}", bufs=1)
    nc.vector.tensor_tensor(out=nt[:], in0=r0[:], in1=r0[:], op=OP.mult)
    nc.vector.tensor_tensor(out=nt[:], in0=nt[:], in1=av_[:], op=OP.mult)
    nc.vector.tensor_scalar(out=nt[:], in0=nt[:], scalar1=-0.5, scalar2=1.5,
                            op0=OP.mult, op1=OP.add)
    r = small.tile([128, EC], F32, tag=f"r_{name}", name=f"r_{name}", bufs=1)
    nc.vector.tensor_tensor(out=r[:], in0=r0[:], in1=nt[:], op=OP.mult)
    s_all = small.tile([128, EC], F32, tag=f"s_{name}", name=f"s_{name}",
                       bufs=1)
    sh_all = small.tile([128, EC], F32, tag=f"sh_{name}", name=f"sh_{name}",
                        bufs=1)
    nc.vector.tensor_tensor(out=s_all[:], in0=r[:], in1=g_sb, op=OP.mult)
    nc.vector.tensor_tensor(out=sh_all[:], in0=mu[:], in1=s_all[:], op=OP.mult)
    nc.vector.tensor_tensor(out=sh_all[:], in0=b_sb, in1=sh_all[:],
                            op=OP.subtract)
    for m in range(EC):
        scales.append(s_all[:, m:m + 1])
        shifts.append(sh_all[:, m:m + 1])
    return scales, shifts


_NC_CACHE = None


def _get_nc():
    global _NC_CACHE
    if _NC_CACHE is None:
        _NC_CACHE = build()
    return _NC_CACHE


def make_in_maps(inputs):
    import ml_dtypes
    f8 = ml_dtypes.float8_e4m3
    bf = ml_dtypes.bfloat16

    shared = {}
    for n in W_NAMES:
        w = np.asarray(inputs[f"{n}_w"], np.float32)       # [E_out, E_in]
        # w8[p, g, i, j] = W[j, (2g+i)*128 + p]
        wt = w.T.reshape(2, 2, 128, E)                     # [g, i, p, j]
        shared[f"{n}_w8"] = np.ascontiguousarray(
            wt.transpose(2, 0, 1, 3)).astype(f8)
    bpk = np.empty((128, len(B_NAMES) * EC), dtype=np.float32)
    for i, n in enumerate(B_NAMES):
        vec = inputs[f"{n}_b"] if n in W_NAMES else inputs[n]
        bpk[:, i * EC:(i + 1) * EC] = np.asarray(vec).reshape(EC, 128).T
    shared["bpk"] = bpk
    sel2 = np.zeros((98, 128), dtype=np.float32)
    for p in range(4):
        sel2[32 * p, 0:64] = 1.0
        sel2[32 * p + 1, 64:128] = 1.0
    shared["sel2"] = sel2.astype(bf)
    shared["idb"] = np.eye(128, dtype=np.float32).astype(bf)
    shared["ones8"] = np.ones((128, 3 * 2 * H * 2),
                               dtype=np.float32).astype(f8)

    x = np.asarray(inputs["x"], dtype=np.float32)
    in_maps = []
    for i in range(N_CORES):
        xc = x[BL * i:BL * (i + 1)]                        # [BL, N, E]
        xT = np.ascontiguousarray(xc.transpose(2, 0, 1).reshape(E, TOK))
        x8 = np.ascontiguousarray(
            xT.reshape(2, 2, 128, TOK).transpose(2, 0, 1, 3)).astype(f8)
        in_maps.append({"x8": x8, "xb": xT.astype(bf), **shared})
    return in_maps


def assemble_output(results):
    y = np.empty((B, N, E), dtype=np.float32)
    for i in range(N_CORES):
        yT = np.asarray(results[i]["yT"]).astype(np.float32)   # [E, TOK]
        y[BL * i:BL * (i + 1)] = yT.reshape(E, BL, N).transpose(1, 2, 0)
    return y


def kernel(**inputs):
    nc = _get_nc()
    in_maps = make_in_maps(inputs)
    res = run_bass_kernel_spmd(nc, in_maps, core_ids=list(range(N_CORES)))
    return assemble_output(res.results)


if __name__ == "__main__":
    nc = build()
    print("build ok")
